# revision 19
# baseline (speedup 1.0000x reference)
"""Restructured GCN kernel: sb-pair-major chunks, staircase, fusion, 4 queues."""
import ml_dtypes
import numpy as np

import concourse.bass as bass
import concourse.bacc as bacc
import concourse.tile as tile
import concourse.mybir as mybir
from concourse.bass_utils import run_bass_kernel_spmd
from concourse.masks import make_identity

dt = mybir.dt
AF = mybir.ActivationFunctionType
OP = mybir.AluOpType
P = 128
QS = [3072, 3072, 3072, 3584]
QO = [0, 3072, 6144, 9216]

CFG = dict(N=100000, E=1600000, B=512, NCORES=8)


def _preprocess(inputs, cfg):
    N, B, ncores = cfg["N"], cfg["B"], cfg["NCORES"]
    nloc = N // ncores
    npad = ((nloc + 511) // 512) * 512
    nblk = npad // P
    BS = 256
    nsb = npad // BS
    if npad == 12800:
        qs, qo = QS, QO
    else:
        qs, qo = [npad], [0]
    nbuck = len(qs)
    src = inputs["edge_index"][0].astype(np.int64)
    dst = inputs["edge_index"][1].astype(np.int64)
    batch = np.asarray(inputs["batch"]).astype(np.int64)
    owner = dst // nloc
    dloc = dst - owner * nloc
    blk = dloc // BS
    sc = src // nloc
    sn = src % nloc
    qs_a = np.asarray(qs); qo_a = np.asarray(qo)
    buck = np.searchsorted(np.cumsum(qs_a), sn, side="right")
    gsrc = (ncores * qo_a[buck] + sc * qs_a[buck]
            + (sn - qo_a[buck])).astype(np.int64)

    order = np.lexsort((gsrc, buck, blk, owner))
    so, sk, sb, sg, sd = (owner[order], buck[order], blk[order], gsrc[order],
                          dloc[order])

    cnt = np.bincount((so * nsb + sb) * nbuck + sk,
                      minlength=ncores * nsb * nbuck
                      ).reshape(ncores, nsb, nbuck)
    NT = ((cnt + P - 1) // P).max(axis=0)  # [nsb, nbuck]

    SBG = 2
    cells = []
    chunks = []
    cell_of_tile = []
    tpos = 0
    for gi_, b0 in enumerate(range(0, nsb, SBG)):
        bs = [b for b in range(b0, min(b0 + SBG, nsb))]
        for k in range(nbuck):
            ck_t0 = tpos
            for b in bs:
                nt = int(NT[b][k])
                if nt == 0:
                    continue
                cells.append((k, b, tpos, nt))
                cell_of_tile += [len(cells) - 1] * nt
                tpos += nt
            if tpos > ck_t0:
                chunks.append((k, ck_t0, tpos - ck_t0, gi_))
    ntile = tpos
    ept = ntile * P
    delay = [0, 2, 4, 5][:nbuck]
    chunks.sort(key=lambda c: (c[3] + delay[c[0]], c[0]))
    chunks = [(k, t0, ntc) for (k, t0, ntc, g) in chunks]

    starts = np.concatenate([[0], np.cumsum(cnt.ravel())[:-1]]).reshape(
        ncores, nsb, nbuck)
    goff = np.zeros((ncores, ept), np.int64)
    dstoff = np.full((ncores, ept), -1.0, np.float32)
    for ci, (k, b, t0, nt) in enumerate(cells):
        p0 = t0 * P
        for c in range(ncores):
            n = int(cnt[c, b, k])
            s0 = int(starts[c, b, k])
            goff[c, p0:p0 + n] = sg[s0:s0 + n] - ncores * qo[k]
            dstoff[c, p0:p0 + n] = (sd[s0:s0 + n] - b * BS).astype(np.float32)

    g16 = goff.reshape(ncores, ept // 16, 16).transpose(0, 2, 1).astype(np.int16)
    g16 = np.tile(g16, (1, 8, 1))
    dst_t = np.ascontiguousarray(
        dstoff.reshape(ncores, ntile, P).transpose(0, 2, 1))

    degf = (np.bincount(dst, minlength=N) + 1).astype(np.float32)
    counts_row = np.bincount(batch, minlength=B).astype(np.float32)[None, :]
    iota = np.broadcast_to(
        np.tile(np.arange(256, dtype=np.float32), 2), (P, 512)).copy()
    ones1 = np.ones((1, P), np.float32)

    x = np.asarray(inputs["x"], np.float32)
    in_maps = []
    for c in range(ncores):
        deg_c = np.ones(npad, np.float32)
        deg_c[:nloc] = degf[c * nloc:(c + 1) * nloc]
        bat_c = np.full(npad, -1, np.int64)
        bat_c[:nloc] = batch[c * nloc:(c + 1) * nloc]
        pool_ind = (
            bat_c.reshape(nblk, P).T[:, :, None]
            == np.arange(B, dtype=np.int64)[None, None, :]
        )
        pool_ind = pool_ind.astype(ml_dtypes.bfloat16).reshape(P, nblk * B)
        xT = np.zeros((P, npad), np.float32)
        xT[:, :nloc] = x[c * nloc:(c + 1) * nloc].T
        in_maps.append({
            "xT": xT,
            "deg_row": deg_c[None, :].copy(),
            "W1": np.asarray(inputs["W1"], np.float32),
            "W2": np.asarray(inputs["W2"], np.float32),
            "lw1": np.asarray(inputs["lw1"], np.float32),
            "lw2": np.asarray(inputs["lw2"], np.float32).reshape(P, 1),
            "b1": np.asarray(inputs["b1"], np.float32).reshape(P, 1),
            "b2": np.asarray(inputs["b2"], np.float32).reshape(P, 1),
            "lb1": np.asarray(inputs["lb1"], np.float32).reshape(P, 1),
            "lb2": np.asarray(inputs["lb2"], np.float32).reshape(1, 1),
            "counts": counts_row,
            "iota": iota,
            "ones1": ones1,
            "gidx": np.ascontiguousarray(g16[c]),
            "dstoff": dst_t[c],
            "pool_ind": np.ascontiguousarray(pool_ind),
        })
    meta = dict(npad=npad, nblk=nblk, nsb=nsb, BS=BS, ntile=ntile, B=B,
                ncores=ncores, cells=cells, cell_of_tile=cell_of_tile,
                chunks=chunks, nbuck=nbuck, qs=qs, qo=qo)
    return in_maps, meta


def _build(m):
    f32, bf16, i16 = dt.float32, dt.bfloat16, dt.int16
    npad, nblk, ntile, B = m["npad"], m["nblk"], m["ntile"], m["B"]
    nsb, BS = m["nsb"], m["BS"]
    ncores = m["ncores"]
    cells, cell_of_tile, chunks = m["cells"], m["cell_of_tile"], m["chunks"]
    groups = [list(range(ncores))]
    qs, qo = m["qs"], m["qo"]
    nq = len(qs)

    nc = bacc.Bacc(None, target_bir_lowering=False, num_swdge_queues=4)
    pr = {}
    for name, shape, d in [
        ("xT", [P, npad], f32), ("deg_row", [1, npad], f32),
        ("W1", [P, P], f32), ("W2", [P, P], f32),
        ("lw1", [P, P], f32), ("lw2", [P, 1], f32), ("b1", [P, 1], f32),
        ("b2", [P, 1], f32), ("lb1", [P, 1], f32), ("lb2", [1, 1], f32),
        ("counts", [1, B], f32), ("iota", [P, 4 * P], f32),
        ("ones1", [1, P], f32),
        ("gidx", [P, ntile * 8], i16), ("dstoff", [P, ntile], f32),
        ("pool_ind", [P, nblk * B], bf16),
    ]:
        pr[name] = nc.declare_dram_parameter(name, shape, d, isOutput=False)
    outp = nc.declare_dram_parameter("out", [1, B], f32, isOutput=True)

    ag_in = [[nc.dram_tensor(f"ag_in{li}_{q}", [qs[q], P], bf16)
              for q in range(nq)] for li in range(2)]
    ag_rep = [[nc.dram_tensor(f"ag_rep{li}_{q}", [ncores * qs[q], P], bf16,
                              addr_space="Shared") for q in range(nq)]
              for li in range(2)]
    ar_in = nc.dram_tensor("ar_in", [P, B], f32)
    ar_out = nc.dram_tensor("ar_out", [P, B], f32, addr_space="Shared")

    with tile.TileContext(nc) as tc:
        with (
            tc.tile_pool(name="pers", bufs=1) as pers,
            tc.tile_pool(name="sml", bufs=1) as sml,
            tc.tile_pool(name="gop", bufs=4) as gop,
            tc.tile_pool(name="dop", bufs=4) as dop,
            tc.tile_pool(name="gbp", bufs=4) as gbp,
            tc.tile_pool(name="indp", bufs=3) as indp,
            tc.tile_pool(name="rowp", bufs=2) as rowp,
            tc.tile_pool(name="dbp", bufs=2) as dbp,
            tc.tile_pool(name="pip", bufs=2) as pip,
            tc.tile_pool(name="psum", bufs=1, space="PSUM") as psp,
        ):
            TA = pers.tile([P, npad], f32)
            TB = pers.tile([P, npad], f32)
            ident = pers.tile([P, P], f32)
            make_identity(nc, ident[:])
            small = {}
            for name, shape, d in [
                ("W1", [P, P], f32), ("W2", [P, P], f32), ("lw1", [P, P], f32),
                ("lw2", [P, 1], f32), ("b1", [P, 1], f32), ("b2", [P, 1], f32),
                ("lb1", [P, 1], f32), ("lb2", [1, 1], f32),
                ("counts", [1, B], f32), ("iota", [P, 4 * P], f32),
                ("ones1", [1, P], f32),
            ]:
                t = sml.tile(shape, d, name=f"sm_{name}")
                nc.sync.dma_start(t[:], pr[name][:])
                small[name] = t
            for q_ in range(nq):
                nc.sync.dma_start(TA[:, qo[q_]:qo[q_] + qs[q_]],
                                  pr["xT"][:, qo[q_]:qo[q_] + qs[q_]])

            dinvR = sml.tile([1, npad], f32)
            nc.sync.dma_start(dinvR[:], pr["deg_row"][:])
            nc.vector.reciprocal(dinvR[:], dinvR[:])
            nc.scalar.activation(dinvR[:], dinvR[:], AF.Sqrt)

            crow = sml.tile([1, B], f32)
            nc.vector.tensor_scalar_max(crow[:], small["counts"][:], 1.0)
            nc.vector.reciprocal(crow[:], crow[:])
            ps = psp.tile([P, B], f32, tag="acc512", bufs=1, name="ps_cnt")
            nc.tensor.matmul(ps[:], small["ones1"][:], crow[:], start=True,
                             stop=True)
            invcnt = sml.tile([P, B], f32)
            nc.vector.tensor_copy(invcnt[:], ps[:])

            pool_state = {"n": 0, "pacc": None}

            def mm_quarter(rhsT, aggT, Wt, li, q):
                for s in range(qo[q] // 512, (qo[q] + qs[q]) // 512):
                    sl = slice(s * 512, (s + 1) * 512)
                    ps1 = psp.tile([P, 512], f32, tag="mm512", bufs=2,
                                   name="ps1")
                    nc.tensor.matmul(ps1[:], Wt[:], rhsT[:, sl],
                                     start=True, stop=True)
                    ps2 = psp.tile([P, 512], f32, tag="bc512", bufs=1,
                                   name="ps2")
                    nc.tensor.matmul(ps2[:], small["ones1"][:],
                                     dinvR[:, sl], start=True, stop=True)
                    db = dbp.tile([P, 512], f32)
                    nc.vector.tensor_copy(db[:], ps2[:])
                    nc.vector.tensor_tensor(aggT[:, sl], ps1[:], db[:],
                                            op=OP.mult)
                    ps3 = psp.tile([P, 512], f32, tag="bc512", bufs=1,
                                   name="ps3")
                    for j in range(4):
                        nc.tensor.transpose(
                            ps3[:, j * P:(j + 1) * P],
                            aggT[:, s * 512 + j * P:s * 512 + (j + 1) * P],
                            ident[:])
                    rows = rowp.tile([P, 512], bf16)
                    nc.scalar.activation(rows[:], ps3[:], AF.Copy)
                    ag_ap = bass.AP(ag_in[li][q], (s * 512 - qo[q]) * P,
                                    [[P, P], [P * P, 4], [1, P]])
                    nc.sync.dma_start(ag_ap, rows[:])
                nc.gpsimd.collective_compute(
                    "AllGather", OP.bypass, replica_groups=groups,
                    ins=[ag_in[li][q][:]], outs=[ag_rep[li][q][:]])

            def extract_sb(b, aggT, outT, bt, final):
                sl = slice(b * BS, (b + 1) * BS)
                ps2 = psp.tile([P, BS], f32, tag="bc256", bufs=1,
                               name="ps2e")
                nc.tensor.matmul(ps2[:], small["ones1"][:], dinvR[:, sl],
                                 start=True, stop=True)
                db = dbp.tile([P, BS], f32, name="db256")
                nc.vector.tensor_copy(db[:], ps2[:])
                tmp = dbp.tile([P, BS], f32, name="tmp256")
                nc.vector.tensor_tensor(tmp[:], aggT[:, sl], db[:],
                                        op=OP.mult)
                nc.scalar.activation(outT[:, sl], tmp[:], AF.Relu,
                                     bias=bt[:, 0:1])
                if final:
                    psr = psp.tile([P, BS], f32, tag="blk", bufs=2,
                                   name="psr")
                    for j in range(2):
                        nc.tensor.transpose(
                            psr[:, j * P:(j + 1) * P],
                            outT[:, b * BS + j * P: b * BS + (j + 1) * P],
                            ident[:])
                    rsb = rowp.tile([P, BS], bf16, name="rsb")
                    nc.vector.tensor_copy(rsb[:], psr[:])
                    for j in range(2):
                        blkid = b * 2 + j
                        pi = pip.tile([P, B], bf16)
                        nc.sync.dma_start(
                            pi[:],
                            pr["pool_ind"][:, blkid * B:(blkid + 1) * B])
                        pool_state["n"] += 1
                        nc.tensor.matmul(
                            pool_state["pacc"][:],
                            rsb[:, j * P:(j + 1) * P], pi[:],
                            start=(pool_state["n"] == 1),
                            stop=(pool_state["n"] == nblk))

            def sweep(li, aggT, outT, bt, final, on_sb_done):
                last_cell_of_sb = {}
                for ci_, (k_, b_, t0_, nt_) in enumerate(cells):
                    last_cell_of_sb[b_] = ci_
                psblk = None
                ntc_max = max(c[2] for c in chunks)
                for ci_ck, (k, t0, ntc) in enumerate(chunks):
                    ch = ntc * P
                    gi = gop.tile([P, ntc_max * 8], i16)
                    nc.sync.dma_start(
                        gi[:, :ntc * 8], pr["gidx"][:, t0 * 8:(t0 + ntc) * 8])
                    do = dop.tile([P, ntc_max], f32)
                    nc.sync.dma_start(
                        do[:, :ntc], pr["dstoff"][:, t0:t0 + ntc])
                    gb = gbp.tile([P, ntc_max, P], bf16)
                    nc.gpsimd.dma_gather(gb[:, :ntc, :], ag_rep[li][k][:],
                                         gi[:, :ntc * 8],
                                         ch, ch, P, single_packet=False,
                                         queue_num=ci_ck % 4)
                    for t4 in range(0, ntc, 2):
                        nb = min(2, ntc - t4)
                        ind2 = indp.tile([P, 2, 2 * P], bf16)
                        iota_ap = bass.AP(small["iota"].tensor, 0,
                                          [[4 * P, P], [2 * P, nb],
                                           [1, 2 * P]])
                        dob = do[:, t4:t4 + nb]
                        do_ap = bass.AP(dob.tensor, dob.offset,
                                        list(dob.ap) + [[0, 2 * P]])
                        nc.vector.tensor_tensor(ind2[:, :nb, :], iota_ap,
                                                do_ap, op=OP.is_equal)
                        for t in range(t4, t4 + nb):
                            gt = t0 + t
                            ci = cell_of_tile[gt]
                            kk, b, ct0, nt = cells[ci]
                            first = gt == ct0
                            last = gt == ct0 + nt - 1
                            if first:
                                psblk = psp.tile([P, BS], f32, tag="blk",
                                                 bufs=2, name="psblk")
                            nc.tensor.matmul(psblk[:], gb[:, t, :],
                                             ind2[:, t - t4, :],
                                             start=first, stop=last)
                            if last:
                                nc.vector.tensor_tensor(
                                    aggT[:, b * BS:(b + 1) * BS],
                                    aggT[:, b * BS:(b + 1) * BS], psblk[:],
                                    op=OP.add)
                                if ci == last_cell_of_sb[b]:
                                    extract_sb(b, aggT, outT, bt, final)
                                    on_sb_done(b)

            for q in range(nq):
                mm_quarter(TA, TB, small["W1"], 0, q)

            sb_to_q = {}
            for q in range(nq):
                for b in range(qo[q] // BS, (qo[q] + qs[q]) // BS):
                    sb_to_q[b] = q
            done_sbs = [set()]

            def on_sb_done_l1(b):
                done_sbs[0].add(b)

            sweep(0, TB, TA, small["b1"], False, on_sb_done_l1)
            for q in range(nq):
                mm_quarter(TA, TB, small["W2"], 1, q)

            pool_state["pacc"] = psp.tile([P, B], f32, tag="acc512", bufs=1,
                                          name="pacc")
            sweep(1, TB, TA, small["b2"], True, lambda b: None)

            pol = sml.tile([P, B], f32)
            nc.vector.tensor_copy(pol[:], pool_state["pacc"][:])
            nc.sync.dma_start(ar_in[:], pol[:])
            nc.gpsimd.collective_compute(
                "AllReduce", OP.add, replica_groups=groups,
                ins=[ar_in[:]], outs=[ar_out[:]])
            pol2 = sml.tile([P, B], f32)
            nc.sync.dma_start(pol2[:], ar_out[:])
            gT = sml.tile([P, B], f32)
            nc.vector.tensor_tensor(gT[:], pol2[:], invcnt[:], op=OP.mult)
            psh = psp.tile([P, B], f32, tag="acc512", bufs=1, name="psh")
            nc.tensor.matmul(psh[:], small["lw1"][:], gT[:], start=True,
                             stop=True)
            z1 = sml.tile([P, B], f32)
            nc.scalar.activation(z1[:], psh[:], AF.Relu,
                                 bias=small["lb1"][:, 0:1])
            pso = psp.tile([1, B], f32, tag="out1", bufs=1, name="pso")
            nc.tensor.matmul(pso[:], small["lw2"][:], z1[:], start=True,
                             stop=True)
            osb = sml.tile([1, B], f32)
            nc.vector.tensor_scalar(osb[:], pso[:], small["lb2"][:1, :1], None,
                                    op0=OP.add)
            nc.sync.dma_start(outp[:], osb[:])
    nc.finalize()
    return nc


def run(inputs, cfg, trace=False):
    in_maps, meta = _preprocess(inputs, cfg)
    nc = _build(meta)
    res = run_bass_kernel_spmd(nc, in_maps, list(range(cfg["NCORES"])),
                               trace=trace)
    out = np.asarray(res.results[0]["out"]).reshape(cfg["B"], 1)
    return out, res


def kernel(**inputs) -> np.ndarray:
    out, _ = run(inputs, CFG)
    return out.astype(np.float32)


# revision 20
# speedup vs baseline: 1.1198x; 1.1198x over previous
"""Restructured GCN kernel: sb-pair-major chunks, staircase, fusion, 4 queues."""
import ml_dtypes
import numpy as np

import concourse.bass as bass
import concourse.bacc as bacc
import concourse.tile as tile
import concourse.mybir as mybir
from concourse.bass_utils import run_bass_kernel_spmd
from concourse.masks import make_identity

dt = mybir.dt
AF = mybir.ActivationFunctionType
OP = mybir.AluOpType
P = 128
QS = [3072, 3072, 3072, 3584]
QO = [0, 3072, 6144, 9216]

CFG = dict(N=100000, E=1600000, B=512, NCORES=8)


def _preprocess(inputs, cfg):
    N, B, ncores = cfg["N"], cfg["B"], cfg["NCORES"]
    nloc = N // ncores
    npad = ((nloc + 511) // 512) * 512
    nblk = npad // P
    BS = 256
    nsb = npad // BS
    if npad == 12800:
        qs, qo = QS, QO
    else:
        qs, qo = [npad], [0]
    nbuck = len(qs)
    src = inputs["edge_index"][0].astype(np.int64)
    dst = inputs["edge_index"][1].astype(np.int64)
    batch = np.asarray(inputs["batch"]).astype(np.int64)
    owner = dst // nloc
    dloc = dst - owner * nloc
    blk = dloc // BS
    sc = src // nloc
    sn = src % nloc
    qs_a = np.asarray(qs); qo_a = np.asarray(qo)
    buck = np.searchsorted(np.cumsum(qs_a), sn, side="right")
    gsrc = (ncores * qo_a[buck] + sc * qs_a[buck]
            + (sn - qo_a[buck])).astype(np.int64)

    order = np.lexsort((gsrc, buck, blk, owner))
    so, sk, sb, sg, sd = (owner[order], buck[order], blk[order], gsrc[order],
                          dloc[order])

    cnt = np.bincount((so * nsb + sb) * nbuck + sk,
                      minlength=ncores * nsb * nbuck
                      ).reshape(ncores, nsb, nbuck)
    NT = ((cnt + P - 1) // P).max(axis=0)  # [nsb, nbuck]

    SBG = 2
    cells = []
    chunks = []
    cell_of_tile = []
    tpos = 0
    for gi_, b0 in enumerate(range(0, nsb, SBG)):
        bs = [b for b in range(b0, min(b0 + SBG, nsb))]
        for k in range(nbuck):
            ck_t0 = tpos
            for b in bs:
                nt = int(NT[b][k])
                if nt == 0:
                    continue
                cells.append((k, b, tpos, nt))
                cell_of_tile += [len(cells) - 1] * nt
                tpos += nt
            if tpos > ck_t0:
                chunks.append((k, ck_t0, tpos - ck_t0, gi_))
    ntile = tpos
    ept = ntile * P
    delay = [0, 2, 4, 6][:nbuck]
    chunks.sort(key=lambda c: (c[3] + delay[c[0]], c[0]))
    chunks = [(k, t0, ntc) for (k, t0, ntc, g) in chunks]

    starts = np.concatenate([[0], np.cumsum(cnt.ravel())[:-1]]).reshape(
        ncores, nsb, nbuck)
    goff = np.zeros((ncores, ept), np.int64)
    dstoff = np.full((ncores, ept), -1.0, np.float32)
    for ci, (k, b, t0, nt) in enumerate(cells):
        p0 = t0 * P
        for c in range(ncores):
            n = int(cnt[c, b, k])
            s0 = int(starts[c, b, k])
            goff[c, p0:p0 + n] = sg[s0:s0 + n] - ncores * qo[k]
            dstoff[c, p0:p0 + n] = (sd[s0:s0 + n] - b * BS).astype(np.float32)

    g16 = goff.reshape(ncores, ept // 16, 16).transpose(0, 2, 1).astype(np.int16)
    g16 = np.tile(g16, (1, 8, 1))
    dst_t = np.ascontiguousarray(
        dstoff.reshape(ncores, ntile, P).transpose(0, 2, 1))

    degf = (np.bincount(dst, minlength=N) + 1).astype(np.float32)
    counts_row = np.bincount(batch, minlength=B).astype(np.float32)[None, :]
    iota = np.broadcast_to(
        np.tile(np.arange(256, dtype=np.float32), 2), (P, 512)).copy()
    ones1 = np.ones((1, P), np.float32)

    x = np.asarray(inputs["x"], np.float32)
    in_maps = []
    for c in range(ncores):
        deg_c = np.ones(npad, np.float32)
        deg_c[:nloc] = degf[c * nloc:(c + 1) * nloc]
        bat_c = np.full(npad, -1, np.int64)
        bat_c[:nloc] = batch[c * nloc:(c + 1) * nloc]
        pool_ind = (
            bat_c.reshape(nblk, P).T[:, :, None]
            == np.arange(B, dtype=np.int64)[None, None, :]
        )
        pool_ind = pool_ind.astype(ml_dtypes.bfloat16).reshape(P, nblk * B)
        xT = np.zeros((P, npad), np.float32)
        xT[:, :nloc] = x[c * nloc:(c + 1) * nloc].T
        in_maps.append({
            "xT": xT,
            "deg_row": deg_c[None, :].copy(),
            "W1": np.asarray(inputs["W1"], np.float32),
            "W2": np.asarray(inputs["W2"], np.float32),
            "lw1": np.asarray(inputs["lw1"], np.float32),
            "lw2": np.asarray(inputs["lw2"], np.float32).reshape(P, 1),
            "b1": np.asarray(inputs["b1"], np.float32).reshape(P, 1),
            "b2": np.asarray(inputs["b2"], np.float32).reshape(P, 1),
            "lb1": np.asarray(inputs["lb1"], np.float32).reshape(P, 1),
            "lb2": np.asarray(inputs["lb2"], np.float32).reshape(1, 1),
            "counts": counts_row,
            "iota": iota,
            "ones1": ones1,
            "gidx": np.ascontiguousarray(g16[c]),
            "dstoff": dst_t[c],
            "pool_ind": np.ascontiguousarray(pool_ind),
        })
    meta = dict(npad=npad, nblk=nblk, nsb=nsb, BS=BS, ntile=ntile, B=B,
                ncores=ncores, cells=cells, cell_of_tile=cell_of_tile,
                chunks=chunks, nbuck=nbuck, qs=qs, qo=qo)
    return in_maps, meta


def _build(m):
    f32, bf16, i16 = dt.float32, dt.bfloat16, dt.int16
    npad, nblk, ntile, B = m["npad"], m["nblk"], m["ntile"], m["B"]
    nsb, BS = m["nsb"], m["BS"]
    ncores = m["ncores"]
    cells, cell_of_tile, chunks = m["cells"], m["cell_of_tile"], m["chunks"]
    groups = [list(range(ncores))]
    qs, qo = m["qs"], m["qo"]
    nq = len(qs)

    nc = bacc.Bacc(None, target_bir_lowering=False, num_swdge_queues=4)
    pr = {}
    for name, shape, d in [
        ("xT", [P, npad], f32), ("deg_row", [1, npad], f32),
        ("W1", [P, P], f32), ("W2", [P, P], f32),
        ("lw1", [P, P], f32), ("lw2", [P, 1], f32), ("b1", [P, 1], f32),
        ("b2", [P, 1], f32), ("lb1", [P, 1], f32), ("lb2", [1, 1], f32),
        ("counts", [1, B], f32), ("iota", [P, 4 * P], f32),
        ("ones1", [1, P], f32),
        ("gidx", [P, ntile * 8], i16), ("dstoff", [P, ntile], f32),
        ("pool_ind", [P, nblk * B], bf16),
    ]:
        pr[name] = nc.declare_dram_parameter(name, shape, d, isOutput=False)
    outp = nc.declare_dram_parameter("out", [1, B], f32, isOutput=True)

    ag_in = [[nc.dram_tensor(f"ag_in{li}_{q}", [qs[q], P], bf16)
              for q in range(nq)] for li in range(2)]
    ag_rep = [[nc.dram_tensor(f"ag_rep{li}_{q}", [ncores * qs[q], P], bf16,
                              addr_space="Shared") for q in range(nq)]
              for li in range(2)]
    ar_in = nc.dram_tensor("ar_in", [P, B], f32)
    ar_out = nc.dram_tensor("ar_out", [P, B], f32, addr_space="Shared")

    with tile.TileContext(nc) as tc:
        with (
            tc.tile_pool(name="pers", bufs=1) as pers,
            tc.tile_pool(name="sml", bufs=1) as sml,
            tc.tile_pool(name="gop", bufs=5) as gop,
            tc.tile_pool(name="dop", bufs=5) as dop,
            tc.tile_pool(name="gbp", bufs=5) as gbp,
            tc.tile_pool(name="indp", bufs=4) as indp,
            tc.tile_pool(name="rowp", bufs=2) as rowp,
            tc.tile_pool(name="dbp", bufs=2) as dbp,
            tc.tile_pool(name="pip", bufs=2) as pip,
            tc.tile_pool(name="psum", bufs=1, space="PSUM") as psp,
        ):
            TA = pers.tile([P, npad], f32)
            TB = pers.tile([P, npad], f32)
            ident = pers.tile([P, P], f32)
            make_identity(nc, ident[:])
            small = {}
            for name, shape, d in [
                ("W1", [P, P], f32), ("W2", [P, P], f32), ("lw1", [P, P], f32),
                ("lw2", [P, 1], f32), ("b1", [P, 1], f32), ("b2", [P, 1], f32),
                ("lb1", [P, 1], f32), ("lb2", [1, 1], f32),
                ("counts", [1, B], f32), ("iota", [P, 4 * P], f32),
                ("ones1", [1, P], f32),
            ]:
                t = sml.tile(shape, d, name=f"sm_{name}")
                nc.sync.dma_start(t[:], pr[name][:])
                small[name] = t
            nc.sync.dma_start(TA[:], pr["xT"][:])

            dinvR = sml.tile([1, npad], f32)
            nc.sync.dma_start(dinvR[:], pr["deg_row"][:])
            nc.vector.reciprocal(dinvR[:], dinvR[:])
            nc.scalar.activation(dinvR[:], dinvR[:], AF.Sqrt)

            crow = sml.tile([1, B], f32)
            nc.vector.tensor_scalar_max(crow[:], small["counts"][:], 1.0)
            nc.vector.reciprocal(crow[:], crow[:])
            ps = psp.tile([P, B], f32, tag="acc512", bufs=1, name="ps_cnt")
            nc.tensor.matmul(ps[:], small["ones1"][:], crow[:], start=True,
                             stop=True)
            invcnt = sml.tile([P, B], f32)
            nc.vector.tensor_copy(invcnt[:], ps[:])

            pool_state = {"n": 0, "pacc": None}

            def mm_quarter(rhsT, aggT, Wt, li, q):
                for s in range(qo[q] // 512, (qo[q] + qs[q]) // 512):
                    sl = slice(s * 512, (s + 1) * 512)
                    ps1 = psp.tile([P, 512], f32, tag="mm512", bufs=2,
                                   name="ps1")
                    nc.tensor.matmul(ps1[:], Wt[:], rhsT[:, sl],
                                     start=True, stop=True)
                    ps2 = psp.tile([P, 512], f32, tag="bc512", bufs=1,
                                   name="ps2")
                    nc.tensor.matmul(ps2[:], small["ones1"][:],
                                     dinvR[:, sl], start=True, stop=True)
                    db = dbp.tile([P, 512], f32)
                    nc.vector.tensor_copy(db[:], ps2[:])
                    nc.vector.tensor_tensor(aggT[:, sl], ps1[:], db[:],
                                            op=OP.mult)
                    ps3 = psp.tile([P, 512], f32, tag="bc512", bufs=1,
                                   name="ps3")
                    for j in range(4):
                        nc.tensor.transpose(
                            ps3[:, j * P:(j + 1) * P],
                            aggT[:, s * 512 + j * P:s * 512 + (j + 1) * P],
                            ident[:])
                    rows = rowp.tile([P, 512], bf16)
                    nc.scalar.activation(rows[:], ps3[:], AF.Copy)
                    ag_ap = bass.AP(ag_in[li][q], (s * 512 - qo[q]) * P,
                                    [[P, P], [P * P, 4], [1, P]])
                    nc.sync.dma_start(ag_ap, rows[:])
                nc.gpsimd.collective_compute(
                    "AllGather", OP.bypass, replica_groups=groups,
                    ins=[ag_in[li][q][:]], outs=[ag_rep[li][q][:]])

            def extract_sb(b, aggT, outT, bt, final):
                sl = slice(b * BS, (b + 1) * BS)
                ps2 = psp.tile([P, BS], f32, tag="bc256", bufs=1,
                               name="ps2e")
                nc.tensor.matmul(ps2[:], small["ones1"][:], dinvR[:, sl],
                                 start=True, stop=True)
                db = dbp.tile([P, BS], f32, name="db256")
                nc.vector.tensor_copy(db[:], ps2[:])
                tmp = dbp.tile([P, BS], f32, name="tmp256")
                nc.vector.tensor_tensor(tmp[:], aggT[:, sl], db[:],
                                        op=OP.mult)
                nc.scalar.activation(outT[:, sl], tmp[:], AF.Relu,
                                     bias=bt[:, 0:1])
                if final:
                    psr = psp.tile([P, BS], f32, tag="blk", bufs=2,
                                   name="psr")
                    for j in range(2):
                        nc.tensor.transpose(
                            psr[:, j * P:(j + 1) * P],
                            outT[:, b * BS + j * P: b * BS + (j + 1) * P],
                            ident[:])
                    rsb = rowp.tile([P, BS], bf16, name="rsb")
                    nc.vector.tensor_copy(rsb[:], psr[:])
                    for j in range(2):
                        blkid = b * 2 + j
                        pi = pip.tile([P, B], bf16)
                        nc.sync.dma_start(
                            pi[:],
                            pr["pool_ind"][:, blkid * B:(blkid + 1) * B])
                        pool_state["n"] += 1
                        nc.tensor.matmul(
                            pool_state["pacc"][:],
                            rsb[:, j * P:(j + 1) * P], pi[:],
                            start=(pool_state["n"] == 1),
                            stop=(pool_state["n"] == nblk))

            def sweep(li, aggT, outT, bt, final, on_sb_done):
                last_cell_of_sb = {}
                for ci_, (k_, b_, t0_, nt_) in enumerate(cells):
                    last_cell_of_sb[b_] = ci_
                psblk = None
                ntc_max = max(c[2] for c in chunks)
                for ci_ck, (k, t0, ntc) in enumerate(chunks):
                    ch = ntc * P
                    gi = gop.tile([P, ntc_max * 8], i16)
                    nc.sync.dma_start(
                        gi[:, :ntc * 8], pr["gidx"][:, t0 * 8:(t0 + ntc) * 8])
                    do = dop.tile([P, ntc_max], f32)
                    nc.sync.dma_start(
                        do[:, :ntc], pr["dstoff"][:, t0:t0 + ntc])
                    gb = gbp.tile([P, ntc_max, P], bf16)
                    nc.gpsimd.dma_gather(gb[:, :ntc, :], ag_rep[li][k][:],
                                         gi[:, :ntc * 8],
                                         ch, ch, P, single_packet=False,
                                         queue_num=ci_ck % 4)
                    for t4 in range(0, ntc, 2):
                        nb = min(2, ntc - t4)
                        ind2 = indp.tile([P, 2, 2 * P], bf16)
                        iota_ap = bass.AP(small["iota"].tensor, 0,
                                          [[4 * P, P], [2 * P, nb],
                                           [1, 2 * P]])
                        dob = do[:, t4:t4 + nb]
                        do_ap = bass.AP(dob.tensor, dob.offset,
                                        list(dob.ap) + [[0, 2 * P]])
                        nc.vector.tensor_tensor(ind2[:, :nb, :], iota_ap,
                                                do_ap, op=OP.is_equal)
                        for t in range(t4, t4 + nb):
                            gt = t0 + t
                            ci = cell_of_tile[gt]
                            kk, b, ct0, nt = cells[ci]
                            first = gt == ct0
                            last = gt == ct0 + nt - 1
                            if first:
                                psblk = psp.tile([P, BS], f32, tag="blk",
                                                 bufs=2, name="psblk")
                            nc.tensor.matmul(psblk[:], gb[:, t, :],
                                             ind2[:, t - t4, :],
                                             start=first, stop=last)
                            if last:
                                nc.vector.tensor_tensor(
                                    aggT[:, b * BS:(b + 1) * BS],
                                    aggT[:, b * BS:(b + 1) * BS], psblk[:],
                                    op=OP.add)
                                if ci == last_cell_of_sb[b]:
                                    extract_sb(b, aggT, outT, bt, final)
                                    on_sb_done(b)

            for q in range(nq):
                mm_quarter(TA, TB, small["W1"], 0, q)

            sb_to_q = {}
            for q in range(nq):
                for b in range(qo[q] // BS, (qo[q] + qs[q]) // BS):
                    sb_to_q[b] = q
            done_sbs = [set()]

            def on_sb_done_l1(b):
                done_sbs[0].add(b)

            sweep(0, TB, TA, small["b1"], False, on_sb_done_l1)
            for q in range(nq):
                mm_quarter(TA, TB, small["W2"], 1, q)

            pool_state["pacc"] = psp.tile([P, B], f32, tag="acc512", bufs=1,
                                          name="pacc")
            sweep(1, TB, TA, small["b2"], True, lambda b: None)

            pol = dbp.tile([P, B], f32, name="pol")
            nc.vector.tensor_copy(pol[:], pool_state["pacc"][:])
            nc.sync.dma_start(ar_in[:], pol[:])
            nc.gpsimd.collective_compute(
                "AllReduce", OP.add, replica_groups=groups,
                ins=[ar_in[:]], outs=[ar_out[:]])
            pol2 = dbp.tile([P, B], f32, name="pol2")
            nc.sync.dma_start(pol2[:], ar_out[:])
            gT = dbp.tile([P, B], f32, name="gT")
            nc.vector.tensor_tensor(gT[:], pol2[:], invcnt[:], op=OP.mult)
            psh = psp.tile([P, B], f32, tag="acc512", bufs=1, name="psh")
            nc.tensor.matmul(psh[:], small["lw1"][:], gT[:], start=True,
                             stop=True)
            z1 = dbp.tile([P, B], f32, name="z1")
            nc.scalar.activation(z1[:], psh[:], AF.Relu,
                                 bias=small["lb1"][:, 0:1])
            pso = psp.tile([1, B], f32, tag="out1", bufs=1, name="pso")
            nc.tensor.matmul(pso[:], small["lw2"][:], z1[:], start=True,
                             stop=True)
            osb = sml.tile([1, B], f32)
            nc.vector.tensor_scalar(osb[:], pso[:], small["lb2"][:1, :1], None,
                                    op0=OP.add)
            nc.sync.dma_start(outp[:], osb[:])
    nc.finalize()
    return nc


def run(inputs, cfg, trace=False):
    in_maps, meta = _preprocess(inputs, cfg)
    nc = _build(meta)
    res = run_bass_kernel_spmd(nc, in_maps, list(range(cfg["NCORES"])),
                               trace=trace)
    out = np.asarray(res.results[0]["out"]).reshape(cfg["B"], 1)
    return out, res


def kernel(**inputs) -> np.ndarray:
    out, _ = run(inputs, CFG)
    return out.astype(np.float32)
